# revision 13
# baseline (speedup 1.0000x reference)
"""TRN2 Bass kernel v3 for nn_NodeEmbedding (3-relation GraphConv + PReLU).

y = PReLU( sum_r (D_in^-1/2 A_r D_out^-1/2 x) W_r + b_r )

Design (per core, 12500 dst nodes = 98 tiles of 128):
  - Edges partitioned by dst-owner core; per (phase=src%4, tile-group) the
    needed x rows for ALL 3 relations are fetched with one dma_gather
    (int16 idx = src//4 into a [25000, 512] fp16 view of x).  Merging the
    relations into one gather (52 instead of 156 instructions) amortizes
    the Q7 descriptor-generation fixed cost, which is the kernel's
    critical path (~8ns/descriptor on one Q7 core pair).
  - Per-core padding slots sit at the bucket tail with idx -1 and
    num_idxs_reg loaded from a per-core count table: the Q7 ucode trims
    trailing negatives and the decode reserves ring space by the register,
    so padding costs no descriptor time on cores with fewer edges than the
    SPMD-common max.  Within a bucket, edges sharing (src, relation, tile)
    share one gather slot.
  - Routing/weighting/aggregation on the PE: for each 128-slot block and
    each (tile, r) it touches, matmul(lhsT=G_block[128 slots x 128 f],
    rhs=Wsel[128 slots x 128 dst]) accumulates aggT_r[f, dst] in PSUM.
    Wsel holds w_e = ns[src]*nd[dst] at (slot, dst%128), zeros elsewhere
    (host-precomputed fp16, per-core data; piece list is the union of all
    cores' occupied cells so the instruction stream is SPMD-common).
  - Phase B inline per tile group: aggT (PSUM) -> SBUF fp16, then
    y = sum_r aggT_r^T W_r + b via PE, PReLU on DVE, sequential store.
"""

import numpy as np
from contextlib import ExitStack

P = 128
F = 128
N_CORES = 8

NPH = 4  # src phases (int16 gather idx = src//4 < 25000)


class Cfg:
    def __init__(self, n_nodes=100000, tg_tiles=8):
        self.N = n_nodes
        self.SHARD = n_nodes // N_CORES
        self.TILES = (self.SHARD + P - 1) // P
        self.NQ = n_nodes // NPH  # quad rows
        self.TGT = tg_tiles
        self.NGRP = (self.TILES + tg_tiles - 1) // tg_tiles


def _schedule(cfg, inputs):
    """Host preprocessing (index-side only).

    Returns (profile, percore) where
      profile[(ph, g)] = dict(nstar=int, pieces=[(b, tl, r), ...])
      percore[c][ph] = dict(idx=[128, cols] int16, wsel=[128, cols] fp16)
    tl = tile index local to the group (0..TGT-1).
    """
    N, SHARD, TILES, TGT, NGRP = cfg.N, cfg.SHARD, cfg.TILES, cfg.TGT, cfg.NGRP
    ns, nd = [], []
    for r in range(3):
        src = np.asarray(inputs[f"src{r}"]).astype(np.int64)
        dst = np.asarray(inputs[f"dst{r}"]).astype(np.int64)
        deg_out = np.bincount(src, minlength=N)
        deg_in = np.bincount(dst, minlength=N)
        ns.append((1.0 / np.sqrt(np.maximum(deg_out, 1))).astype(np.float32))
        nd.append((1.0 / np.sqrt(np.maximum(deg_in, 1))).astype(np.float32))

    # per core: merged edge arrays sorted by (ph, g, r, tile, q); within a
    # bucket (ph, g), edges sharing (r, tile, src) share one gather slot
    NQ = cfg.NQ
    bdata = {}
    cnt = np.zeros((N_CORES, NPH, NGRP), np.int64)
    for c in range(N_CORES):
        qs, tls, dms, ws, phs, rs = [], [], [], [], [], []
        for r in range(3):
            src = np.asarray(inputs[f"src{r}"]).astype(np.int64)
            dst = np.asarray(inputs[f"dst{r}"]).astype(np.int64)
            m = (dst // SHARD) == c
            s, d = src[m], dst[m] - c * SHARD
            w = ns[r][s] * nd[r][dst[m]]
            qs.append(s // NPH)
            phs.append(s % NPH)
            tls.append(d // P)
            dms.append(d % P)
            ws.append(w)
            rs.append(np.full(s.shape[0], r, np.int64))
        q = np.concatenate(qs)
        ph = np.concatenate(phs)
        tile = np.concatenate(tls)
        dm = np.concatenate(dms)
        w = np.concatenate(ws).astype(np.float32)
        r_ = np.concatenate(rs)
        g = tile // TGT
        bid = ph * NGRP + g
        order = np.argsort(bid, kind="stable")
        bid, q, tile, dm, w, r_ = (a[order] for a in (bid, q, tile, dm, w, r_))
        bounds = np.searchsorted(bid, np.arange(NPH * NGRP + 1))
        for ph_ in range(NPH):
            for g_ in range(NGRP):
                e0, e1 = bounds[ph_ * NGRP + g_], bounds[ph_ * NGRP + g_ + 1]
                tl = tile[e0:e1] - g_ * TGT
                # slot key sorts by (r, tile, q)
                ek = (r_[e0:e1] * TGT + tl) * NQ + q[e0:e1]
                uk, inv = np.unique(ek, return_inverse=True)
                cnt[c, ph_, g_] = len(uk)
                bdata[c, ph_, g_] = (uk, inv, dm[e0:e1], w[e0:e1])

    nstar = ((cnt.max(0) + 15) // 16) * 16  # [NPH, NGRP]

    profile = {}
    percore = [dict() for _ in range(N_CORES)]
    keyspace = TGT * 3
    for ph in range(NPH):
        for g in range(NGRP):
            nst = int(nstar[ph, g])
            # union of occupied (block, tl, r) cells over cores; cell key
            # encodes (block, tl, r) so sorted keys give (b, tl, r) order
            core_info = []
            allkeys = []
            for c in range(N_CORES):
                uk, inv, dmm, wm = bdata[c, ph, g]
                tlr = uk // NQ
                r_of = tlr // TGT
                tl_of = tlr % TGT
                blk_of = np.arange(len(uk)) // P
                cell_of_slot = blk_of * keyspace + tl_of * 3 + r_of
                cellkey = cell_of_slot[inv]
                allkeys.append(np.unique(cellkey))
                core_info.append((uk % NQ, inv, cellkey, dmm, wm))
            ukeys = np.unique(np.concatenate(allkeys))
            pieces = [(int(k) // keyspace, (int(k) % keyspace) // 3,
                       int(k) % 3) for k in ukeys]
            parr = np.zeros(int(ukeys[-1]) + 1 if len(ukeys) else 1, np.int64)
            parr[ukeys] = np.arange(len(ukeys))
            profile[ph, g] = dict(nstar=nst, pieces=pieces)
            for c in range(N_CORES):
                q_of_slot, inv, cellkey, dmm, wm = core_info[c]
                idx_flat = np.full(nst, -1, np.int16)
                idx_flat[:len(q_of_slot)] = q_of_slot.astype(np.int16)
                wsel32 = np.zeros((P, max(len(pieces), 1) * P), np.float32)
                np.add.at(wsel32, (inv % P, parr[cellkey] * P + dmm), wm)
                percore[c].setdefault(ph, {})[g] = (
                    idx_flat, wsel32.astype(np.float16))

    # concatenate per-ph idx and wsel arrays with per-g column offsets
    for c in range(N_CORES):
        for ph in range(NPH):
            idxs = []
            wsels = []
            for g in range(NGRP):
                idx_flat, wsel = percore[c][ph][g]
                nst = idx_flat.shape[0]
                wrap = np.zeros((16, nst // 16), np.int16)
                ar = np.arange(nst)
                wrap[ar % 16, ar // 16] = idx_flat
                idxs.append(wrap)
                wsels.append(wsel)
            idx_cat = np.concatenate(idxs, axis=1)
            percore[c][ph] = dict(
                idx=np.tile(idx_cat, (8, 1)),
                wsel=np.concatenate(wsels, axis=1))
        percore[c]["cnts"] = cnt[c].reshape(1, NPH * NGRP).astype(np.int32)
    return profile, percore


def _build_bass(cfg, profile):
    import concourse.bass as bass
    import concourse.bacc as bacc
    import concourse.tile as tile
    import concourse.mybir as mybir

    TILES, SHARD, TGT, NGRP = cfg.TILES, cfg.SHARD, cfg.TGT, cfg.NGRP
    dt = mybir.dt

    nc = bacc.Bacc("TRN2", target_bir_lowering=False, debug=False,
                   num_devices=N_CORES)

    x4h = nc.dram_tensor("x4h", [cfg.NQ, NPH * F], dt.float16,
                         kind="ExternalInput")
    idx_t, wsel_t = {}, {}
    idxcols = {ph: sum(profile[ph, g]["nstar"] for g in range(NGRP)) // 16
               for ph in range(NPH)}
    wselcols = {ph: sum(max(len(profile[ph, g]["pieces"]), 1)
                        for g in range(NGRP)) * P for ph in range(NPH)}
    for ph in range(NPH):
        idx_t[ph] = nc.dram_tensor(f"idx{ph}", [P, idxcols[ph]], dt.int16,
                                   kind="ExternalInput")
        wsel_t[ph] = nc.dram_tensor(f"wsel{ph}", [P, wselcols[ph]],
                                    dt.float16, kind="ExternalInput")
    W_t = [nc.dram_tensor(f"W{r}", [F, F], dt.float16, kind="ExternalInput")
           for r in range(3)]
    b_t = nc.dram_tensor("bvec", [1, 512], dt.float16, kind="ExternalInput")
    pa_t = nc.dram_tensor("prelu_a", [1], dt.float32, kind="ExternalInput")
    cnt_t = nc.dram_tensor("cnts", [1, NPH * NGRP], dt.int32,
                           kind="ExternalInput")
    y_t = nc.dram_tensor("y", [TILES * P, F], dt.float32,
                         kind="ExternalOutput")

    max_nb = 0
    max_pieces = 1
    for key, pr in profile.items():
        max_nb = max(max_nb, (pr["nstar"] + P - 1) // P)
        max_pieces = max(max_pieces, len(pr["pieces"]))

    with tile.TileContext(nc) as tc:
        with ExitStack() as ctx:
            cpool = ctx.enter_context(tc.tile_pool(name="const", bufs=1))
            ipool = ctx.enter_context(tc.tile_pool(name="idx", bufs=3))
            gpool = ctx.enter_context(tc.tile_pool(name="g", bufs=2))
            wpool = ctx.enter_context(tc.tile_pool(name="wsel", bufs=2))
            apool = ctx.enter_context(tc.tile_pool(name="aggsb", bufs=2))
            ypool = ctx.enter_context(tc.tile_pool(name="y", bufs=2))

            # constants
            W_sb = []
            for r in range(3):
                w_ = cpool.tile([F, F], dt.float16, tag=f"W{r}")
                nc.sync.dma_start(w_[:], W_t[r][:, :])
                W_sb.append(w_)
            b_sb = cpool.tile([1, 512], dt.float16)
            nc.sync.dma_start(b_sb[:], b_t[:, :])
            ones1 = cpool.tile([1, P], dt.float16)
            nc.vector.memset(ones1[:], 1.0)
            ones1f = cpool.tile([1, P], dt.float32)
            nc.vector.memset(ones1f[:], 1.0)
            zvec = cpool.tile([P, 512], dt.float16)
            nc.vector.memset(zvec[:], 0.0)
            pa_sb = cpool.tile([1, 1], dt.float32)
            nc.sync.dma_start(pa_sb[:], pa_t[None, :])
            cnt_sb = cpool.tile([1, NPH * NGRP], dt.int32)
            nc.sync.dma_start(cnt_sb[:], cnt_t[:, :])
            am1 = cpool.tile([P, 1], dt.float32)
            with tc.tile_pool(name="ppsum", bufs=1, space="PSUM") as ppool:
                pa_ps = ppool.tile([P, 1], dt.float32, space="PSUM")
                nc.tensor.matmul(pa_ps[:], lhsT=ones1f[:], rhs=pa_sb[:],
                                 start=True, stop=True)
                nc.vector.tensor_scalar_add(am1[:], pa_ps[:], -1.0)

            pagg = ctx.enter_context(
                tc.tile_pool(name="pagg", bufs=1, space="PSUM"))
            py_pool = ctx.enter_context(
                tc.tile_pool(name="py", bufs=2, space="PSUM"))

            # zero G pool slots once: pieces may read rows no gather wrote
            # (per-core counts < common max); raw SBUF can be NaN patterns
            # and PE NaN*0 = NaN.
            for _ in range(2):
                gz = gpool.tile([P, max_nb, F], dt.float16, tag="G")
                nc.vector.memset(gz[:], 0.0)

            idx_off = {ph: 0 for ph in range(NPH)}
            wsel_off = {ph: 0 for ph in range(NPH)}
            for g in range(NGRP):
                t0 = g * TGT
                t1 = min(t0 + TGT, TILES)
                ntl = t1 - t0
                nbank = (ntl + 3) // 4
                aggps = {}
                for r in range(3):
                    for bk in range(nbank):
                        ps = pagg.tile([P, 512], dt.float32, space="PSUM",
                                       tag=f"agg{r}_{bk}")
                        # zero the bank and set accumulate bits everywhere
                        nc.tensor.matmul(ps[:, :], lhsT=zvec[:, 0:P],
                                         rhs=zvec[:, 0:512], start=True,
                                         stop=False, skip_group_check=True)
                        aggps[r, bk] = ps
                for ph in range(NPH):
                    pr = profile[ph, g]
                    n = pr["nstar"]
                    pieces = pr["pieces"]
                    o16 = idx_off[ph]
                    po = wsel_off[ph]
                    idx_off[ph] += n // 16
                    wsel_off[ph] += max(len(pieces), 1) * P
                    if n == 0:
                        continue
                    nb = (n + P - 1) // P
                    it = ipool.tile([P, n // 16], dt.int16, tag="idx")
                    nc.sync.dma_start(it[:], idx_t[ph][:, o16:o16 + n // 16])
                    G = gpool.tile([P, max_nb, F], dt.float16, tag="G")
                    k = ph * NGRP + g
                    nv = nc.gpsimd.value_load(
                        cnt_sb[0:1, k:k + 1], min_val=0, max_val=n)
                    nc.gpsimd.dma_gather(
                        out_ap=G[:, 0:nb, :],
                        in_ap=x4h[:, ph * F:(ph + 1) * F],
                        idxs_ap=it[:, :],
                        num_idxs=n, num_idxs_reg=nv, elem_size=F,
                        elem_step=NPH * F,
                        single_packet=(n <= 1024))
                    if pieces:
                        wse = wpool.tile([P, max_pieces * P], dt.float16,
                                         tag="wsel")
                        nc.sync.dma_start(
                            wse[:, 0:len(pieces) * P],
                            wsel_t[ph][:, po:po + len(pieces) * P])
                        for k, (b, tl, r) in enumerate(pieces):
                            bk, c0 = tl // 4, (tl % 4) * P
                            nc.tensor.matmul(
                                aggps[r, bk][:, c0:c0 + P],
                                lhsT=G[:, b, :],
                                rhs=wse[:, k * P:(k + 1) * P],
                                start=False, stop=False,
                                skip_group_check=True)
                # ---- phase B for this tile group ----
                aggsb = {}
                for r in range(3):
                    for bk in range(nbank):
                        asb = apool.tile([P, 512], dt.float16,
                                         tag=f"as{r}_{bk}")
                        nc.vector.tensor_copy(asb[:], aggps[r, bk][:, :])
                        aggsb[r, bk] = asb
                for bk in range(nbank):
                    yps = py_pool.tile([P, 512], dt.float32, space="PSUM",
                                       tag="yps")
                    nc.tensor.matmul(yps[:, :], lhsT=ones1[:], rhs=b_sb[:, :],
                                     start=True, stop=False,
                                     skip_group_check=True)
                    for tl4 in range(min(4, ntl - bk * 4)):
                        c0 = tl4 * P
                        for r in range(3):
                            nc.tensor.matmul(
                                yps[:, c0:c0 + P],
                                lhsT=aggsb[r, bk][:, c0:c0 + P],
                                rhs=W_sb[r][:, :],
                                start=False, stop=False,
                                skip_group_check=True)
                    neg = ypool.tile([P, 512], dt.float32, tag="neg")
                    nc.vector.tensor_scalar_min(neg[:], yps[:, :], 0.0)
                    ysb = ypool.tile([P, 512], dt.float32, tag="ysb")
                    nc.vector.scalar_tensor_tensor(
                        out=ysb[:], in0=neg[:], scalar=am1[:, :1],
                        in1=yps[:, :],
                        op0=mybir.AluOpType.mult,
                        op1=mybir.AluOpType.add)
                    for tl4 in range(min(4, ntl - bk * 4)):
                        t = t0 + bk * 4 + tl4
                        nc.sync.dma_start(
                            y_t[t * P:(t + 1) * P, :],
                            ysb[:, tl4 * P:(tl4 + 1) * P])

    nc.compile()
    return nc


_NC_CACHE = {}


def _profile_key(profile):
    import hashlib
    h = hashlib.sha256()
    for k in sorted(profile):
        pr = profile[k]
        h.update(repr((k, pr["nstar"], pr["pieces"])).encode())
    return h.hexdigest()


def _run(cfg, inputs, trace=False, trace_kwargs=None):
    from concourse.bass_utils import run_bass_kernel_spmd

    x = np.ascontiguousarray(np.asarray(inputs["x"], dtype=np.float32))
    profile, percore = _schedule(cfg, inputs)
    key = (cfg.N, cfg.TGT, _profile_key(profile))
    nc = _NC_CACHE.get(key)
    if nc is None:
        nc = _build_bass(cfg, profile)
        _NC_CACHE.clear()
        _NC_CACHE[key] = nc

    x4h = x.astype(np.float16).reshape(cfg.NQ, NPH * F)
    bsum = (np.asarray(inputs["b0"]) + np.asarray(inputs["b1"])
            + np.asarray(inputs["b2"])).astype(np.float16)
    bvec = np.tile(bsum, 4)[None, :]
    in_maps = []
    for c in range(N_CORES):
        m = {"x4h": x4h, "bvec": bvec,
             "prelu_a": np.asarray(inputs["prelu_a"], dtype=np.float32),
             "cnts": percore[c]["cnts"]}
        for r in range(3):
            m[f"W{r}"] = np.asarray(inputs[f"W{r}"],
                                    dtype=np.float32).astype(np.float16)
        for ph in range(NPH):
            m[f"idx{ph}"] = percore[c][ph]["idx"]
            m[f"wsel{ph}"] = percore[c][ph]["wsel"]
        in_maps.append(m)

    res = run_bass_kernel_spmd(nc, in_maps, core_ids=list(range(N_CORES)),
                               trace=trace, **(trace_kwargs or {}))
    y = np.concatenate(
        [res.results[c]["y"][:cfg.SHARD] for c in range(N_CORES)], axis=0)
    return y, res


def kernel(**inputs) -> np.ndarray:
    cfg = Cfg()
    y, _ = _run(cfg, inputs)
    return y.astype(np.float32)


if __name__ == "__main__":
    pass


# revision 21
# speedup vs baseline: 1.7510x; 1.7510x over previous
"""TRN2 Bass kernel v3 for nn_NodeEmbedding (3-relation GraphConv + PReLU).

y = PReLU( sum_r (D_in^-1/2 A_r D_out^-1/2 x) W_r + b_r )

Design (per core, 12500 dst nodes = 98 tiles of 128):
  - Edges partitioned by dst-owner core; per (phase=src%4, tile-group) the
    needed x rows for ALL 3 relations are fetched with one dma_gather
    (int16 idx = src//4 into a [25000, 512] fp16 view of x).  Merging the
    relations into one gather (52 instead of 156 instructions) amortizes
    the Q7 descriptor-generation fixed cost, which is the kernel's
    critical path (~8ns/descriptor on one Q7 core pair).
  - Per-core padding slots sit at the bucket tail with idx -1 and
    num_idxs_reg loaded from a per-core count table: the Q7 ucode trims
    trailing negatives and the decode reserves ring space by the register,
    so padding costs no descriptor time on cores with fewer edges than the
    SPMD-common max.  Within a bucket, edges sharing (src, relation, tile)
    share one gather slot.
  - Routing/weighting/aggregation on the PE: for each 128-slot block and
    each (tile, r) it touches, matmul(lhsT=G_block[128 slots x 128 f],
    rhs=Wsel[128 slots x 128 dst]) accumulates aggT_r[f, dst] in PSUM.
    Wsel holds w_e = ns[src]*nd[dst] at (slot, dst%128), zeros elsewhere
    (host-precomputed fp16, per-core data; piece list is the union of all
    cores' occupied cells so the instruction stream is SPMD-common).
  - Phase B inline per tile group: aggT (PSUM) -> SBUF fp16, then
    y = sum_r aggT_r^T W_r + b via PE, PReLU on DVE, sequential store.
"""

import numpy as np
from contextlib import ExitStack

P = 128
F = 128
N_CORES = 8

NPH = 4  # src phases (int16 gather idx = src//4 < 25000)
USE_TRIM = False  # per-core num_idxs_reg trim of bucket tail padding


class Cfg:
    def __init__(self, n_nodes=100000, tg_tiles=8):
        self.N = n_nodes
        self.SHARD = n_nodes // N_CORES
        self.TILES = (self.SHARD + P - 1) // P
        self.NQ = n_nodes // NPH  # quad rows
        self.TGT = tg_tiles
        self.NGRP = (self.TILES + tg_tiles - 1) // tg_tiles


def _schedule(cfg, inputs):
    """Host preprocessing (index-side only).

    Returns (profile, percore) where
      profile[(ph, g)] = dict(nstar=int, pieces=[(b, tl, r), ...])
      percore[c][ph] = dict(idx=[128, cols] int16, wsel=[128, cols] fp16)
    tl = tile index local to the group (0..TGT-1).
    """
    N, SHARD, TILES, TGT, NGRP = cfg.N, cfg.SHARD, cfg.TILES, cfg.TGT, cfg.NGRP
    ns, nd = [], []
    for r in range(3):
        src = np.asarray(inputs[f"src{r}"]).astype(np.int64)
        dst = np.asarray(inputs[f"dst{r}"]).astype(np.int64)
        deg_out = np.bincount(src, minlength=N)
        deg_in = np.bincount(dst, minlength=N)
        ns.append((1.0 / np.sqrt(np.maximum(deg_out, 1))).astype(np.float32))
        nd.append((1.0 / np.sqrt(np.maximum(deg_in, 1))).astype(np.float32))

    # per core: merged edge arrays sorted by (ph, g, r, tile, q); within a
    # bucket (ph, g), edges sharing (r, tile, src) share one gather slot
    NQ = cfg.NQ
    bdata = {}
    cnt = np.zeros((N_CORES, NPH, NGRP), np.int64)
    for c in range(N_CORES):
        qs, tls, dms, ws, phs, rs = [], [], [], [], [], []
        for r in range(3):
            src = np.asarray(inputs[f"src{r}"]).astype(np.int64)
            dst = np.asarray(inputs[f"dst{r}"]).astype(np.int64)
            m = (dst // SHARD) == c
            s, d = src[m], dst[m] - c * SHARD
            w = ns[r][s] * nd[r][dst[m]]
            qs.append(s // NPH)
            phs.append(s % NPH)
            tls.append(d // P)
            dms.append(d % P)
            ws.append(w)
            rs.append(np.full(s.shape[0], r, np.int64))
        q = np.concatenate(qs)
        ph = np.concatenate(phs)
        tile = np.concatenate(tls)
        dm = np.concatenate(dms)
        w = np.concatenate(ws).astype(np.float32)
        r_ = np.concatenate(rs)
        g = tile // TGT
        bid = ph * NGRP + g
        order = np.argsort(bid, kind="stable")
        bid, q, tile, dm, w, r_ = (a[order] for a in (bid, q, tile, dm, w, r_))
        bounds = np.searchsorted(bid, np.arange(NPH * NGRP + 1))
        for ph_ in range(NPH):
            for g_ in range(NGRP):
                e0, e1 = bounds[ph_ * NGRP + g_], bounds[ph_ * NGRP + g_ + 1]
                tl = tile[e0:e1] - g_ * TGT
                # slot key sorts by (r, tile, q)
                ek = (r_[e0:e1] * TGT + tl) * NQ + q[e0:e1]
                uk, inv = np.unique(ek, return_inverse=True)
                cnt[c, ph_, g_] = len(uk)
                bdata[c, ph_, g_] = (uk, inv, dm[e0:e1], w[e0:e1])

    nstar = ((cnt.max(0) + 15) // 16) * 16  # [NPH, NGRP]

    profile = {}
    percore = [dict() for _ in range(N_CORES)]
    keyspace = TGT * 3
    for ph in range(NPH):
        for g in range(NGRP):
            nst = int(nstar[ph, g])
            # union of occupied (block, tl, r) cells over cores; cell key
            # encodes (block, tl, r) so sorted keys give (b, tl, r) order
            core_info = []
            allkeys = []
            for c in range(N_CORES):
                uk, inv, dmm, wm = bdata[c, ph, g]
                tlr = uk // NQ
                r_of = tlr // TGT
                tl_of = tlr % TGT
                blk_of = np.arange(len(uk)) // P
                cell_of_slot = blk_of * keyspace + tl_of * 3 + r_of
                cellkey = cell_of_slot[inv]
                allkeys.append(np.unique(cellkey))
                core_info.append((uk % NQ, inv, cellkey, dmm, wm))
            ukeys = np.unique(np.concatenate(allkeys))
            pieces = [(int(k) // keyspace, (int(k) % keyspace) // 3,
                       int(k) % 3) for k in ukeys]
            parr = np.zeros(int(ukeys[-1]) + 1 if len(ukeys) else 1, np.int64)
            parr[ukeys] = np.arange(len(ukeys))
            profile[ph, g] = dict(nstar=nst, pieces=pieces)
            for c in range(N_CORES):
                q_of_slot, inv, cellkey, dmm, wm = core_info[c]
                idx_flat = np.full(nst, -1 if USE_TRIM else 0, np.int16)
                idx_flat[:len(q_of_slot)] = q_of_slot.astype(np.int16)
                wsel32 = np.zeros((P, max(len(pieces), 1) * P), np.float32)
                np.add.at(wsel32, (inv % P, parr[cellkey] * P + dmm), wm)
                percore[c].setdefault(ph, {})[g] = (
                    idx_flat, wsel32.astype(np.float16))

    # concatenate per-ph idx and wsel arrays with per-g column offsets
    for c in range(N_CORES):
        for ph in range(NPH):
            idxs = []
            wsels = []
            for g in range(NGRP):
                idx_flat, wsel = percore[c][ph][g]
                nst = idx_flat.shape[0]
                wrap = np.zeros((16, nst // 16), np.int16)
                ar = np.arange(nst)
                wrap[ar % 16, ar // 16] = idx_flat
                idxs.append(wrap)
                wsels.append(wsel)
            idx_cat = np.concatenate(idxs, axis=1)
            percore[c][ph] = dict(
                idx=np.tile(idx_cat, (8, 1)),
                wsel=np.concatenate(wsels, axis=1))
        percore[c]["cnts"] = cnt[c].reshape(1, NPH * NGRP).astype(np.int32)
    return profile, percore


def _build_bass(cfg, profile):
    import concourse.bass as bass
    import concourse.bacc as bacc
    import concourse.tile as tile
    import concourse.mybir as mybir

    TILES, SHARD, TGT, NGRP = cfg.TILES, cfg.SHARD, cfg.TGT, cfg.NGRP
    dt = mybir.dt

    nc = bacc.Bacc("TRN2", target_bir_lowering=False, debug=False,
                   num_devices=N_CORES, num_swdge_queues=4)

    x4h = nc.dram_tensor("x4h", [cfg.NQ, NPH * F], dt.float16,
                         kind="ExternalInput")
    idx_t, wsel_t = {}, {}
    idxcols = {ph: sum(profile[ph, g]["nstar"] for g in range(NGRP)) // 16
               for ph in range(NPH)}
    wselcols = {ph: sum(max(len(profile[ph, g]["pieces"]), 1)
                        for g in range(NGRP)) * P for ph in range(NPH)}
    for ph in range(NPH):
        idx_t[ph] = nc.dram_tensor(f"idx{ph}", [P, idxcols[ph]], dt.int16,
                                   kind="ExternalInput")
        wsel_t[ph] = nc.dram_tensor(f"wsel{ph}", [P, wselcols[ph]],
                                    dt.float16, kind="ExternalInput")
    W_t = [nc.dram_tensor(f"W{r}", [F, F], dt.float16, kind="ExternalInput")
           for r in range(3)]
    b_t = nc.dram_tensor("bvec", [1, 512], dt.float16, kind="ExternalInput")
    pa_t = nc.dram_tensor("prelu_a", [1], dt.float32, kind="ExternalInput")
    cnt_t = nc.dram_tensor("cnts", [1, NPH * NGRP], dt.int32,
                           kind="ExternalInput")
    y_t = nc.dram_tensor("y", [TILES * P, F], dt.float32,
                         kind="ExternalOutput")

    max_nb = 0
    max_pieces = 1
    for key, pr in profile.items():
        max_nb = max(max_nb, (pr["nstar"] + P - 1) // P)
        max_pieces = max(max_pieces, len(pr["pieces"]))

    with tile.TileContext(nc) as tc:
        with ExitStack() as ctx:
            cpool = ctx.enter_context(tc.tile_pool(name="const", bufs=1))
            ipool = ctx.enter_context(tc.tile_pool(name="idx", bufs=3))
            gpool = ctx.enter_context(tc.tile_pool(name="g", bufs=2))
            wpool = ctx.enter_context(tc.tile_pool(name="wsel", bufs=2))
            apool = ctx.enter_context(tc.tile_pool(name="aggsb", bufs=2))
            ypool = ctx.enter_context(tc.tile_pool(name="y", bufs=2))

            # constants
            W_sb = []
            for r in range(3):
                w_ = cpool.tile([F, F], dt.float16, tag=f"W{r}")
                nc.sync.dma_start(w_[:], W_t[r][:, :])
                W_sb.append(w_)
            b_sb = cpool.tile([1, 512], dt.float16)
            nc.sync.dma_start(b_sb[:], b_t[:, :])
            ones1 = cpool.tile([1, P], dt.float16)
            nc.vector.memset(ones1[:], 1.0)
            ones1f = cpool.tile([1, P], dt.float32)
            nc.vector.memset(ones1f[:], 1.0)
            zvec = cpool.tile([P, 512], dt.float16)
            nc.vector.memset(zvec[:], 0.0)
            pa_sb = cpool.tile([1, 1], dt.float32)
            nc.sync.dma_start(pa_sb[:], pa_t[None, :])
            cnt_sb = cpool.tile([1, NPH * NGRP], dt.int32)
            nc.sync.dma_start(cnt_sb[:], cnt_t[:, :])
            am1 = cpool.tile([P, 1], dt.float32)
            with tc.tile_pool(name="ppsum", bufs=1, space="PSUM") as ppool:
                pa_ps = ppool.tile([P, 1], dt.float32, space="PSUM")
                nc.tensor.matmul(pa_ps[:], lhsT=ones1f[:], rhs=pa_sb[:],
                                 start=True, stop=True)
                nc.vector.tensor_scalar_add(am1[:], pa_ps[:], -1.0)

            pagg = ctx.enter_context(
                tc.tile_pool(name="pagg", bufs=1, space="PSUM"))
            py_pool = ctx.enter_context(
                tc.tile_pool(name="py", bufs=2, space="PSUM"))

            # zero G pool slots once: pieces may read rows no gather wrote
            # (per-core counts < common max); raw SBUF can be NaN patterns
            # and PE NaN*0 = NaN.
            for _ in range(2):
                gz = gpool.tile([P, max_nb, F], dt.float16, tag="G")
                nc.vector.memset(gz[:], 0.0)

            # TRIM_REGS: preload per-bucket runtime counts into Pool
            # registers so no sequencer op sits between gathers (the Pool
            # engine's exec window is what lets gathers on different SWDGE
            # queues overlap on different Q7 core pairs)
            cnt_reg = {}
            if USE_TRIM:
                for g in range(NGRP):
                    for ph in range(NPH):
                        k = ph * NGRP + g
                        cnt_reg[ph, g] = nc.gpsimd.value_load(
                            cnt_sb[0:1, k:k + 1], min_val=0,
                            max_val=profile[ph, g]["nstar"])

            idx_off = {ph: 0 for ph in range(NPH)}
            wsel_off = {ph: 0 for ph in range(NPH)}
            for g in range(NGRP):
                t0 = g * TGT
                t1 = min(t0 + TGT, TILES)
                ntl = t1 - t0
                nbank = (ntl + 3) // 4
                aggps = {}
                for r in range(3):
                    for bk in range(nbank):
                        ps = pagg.tile([P, 512], dt.float32, space="PSUM",
                                       tag=f"agg{r}_{bk}")
                        # zero the bank and set accumulate bits everywhere
                        nc.tensor.matmul(ps[:, :], lhsT=zvec[:, 0:P],
                                         rhs=zvec[:, 0:512], start=True,
                                         stop=False, skip_group_check=True)
                        aggps[r, bk] = ps
                for ph in range(NPH):
                    pr = profile[ph, g]
                    n = pr["nstar"]
                    pieces = pr["pieces"]
                    o16 = idx_off[ph]
                    po = wsel_off[ph]
                    idx_off[ph] += n // 16
                    wsel_off[ph] += max(len(pieces), 1) * P
                    if n == 0:
                        continue
                    nb = (n + P - 1) // P
                    it = ipool.tile([P, n // 16], dt.int16, tag="idx")
                    nc.sync.dma_start(it[:], idx_t[ph][:, o16:o16 + n // 16])
                    G = gpool.tile([P, max_nb, F], dt.float16, tag="G")
                    nc.gpsimd.dma_gather(
                        out_ap=G[:, 0:nb, :],
                        in_ap=x4h[:, ph * F:(ph + 1) * F],
                        idxs_ap=it[:, :],
                        num_idxs=n,
                        num_idxs_reg=cnt_reg[ph, g] if USE_TRIM else n,
                        elem_size=F,
                        elem_step=NPH * F,
                        single_packet=(n <= 1024),
                        queue_num=(g * NPH + ph) % 4)
                    if pieces:
                        wse = wpool.tile([P, max_pieces * P], dt.float16,
                                         tag="wsel")
                        nc.sync.dma_start(
                            wse[:, 0:len(pieces) * P],
                            wsel_t[ph][:, po:po + len(pieces) * P])
                        for k, (b, tl, r) in enumerate(pieces):
                            bk, c0 = tl // 4, (tl % 4) * P
                            nc.tensor.matmul(
                                aggps[r, bk][:, c0:c0 + P],
                                lhsT=G[:, b, :],
                                rhs=wse[:, k * P:(k + 1) * P],
                                start=False, stop=False,
                                skip_group_check=True)
                # ---- phase B for this tile group ----
                aggsb = {}
                for r in range(3):
                    for bk in range(nbank):
                        asb = apool.tile([P, 512], dt.float16,
                                         tag=f"as{r}_{bk}")
                        nc.vector.tensor_copy(asb[:], aggps[r, bk][:, :])
                        aggsb[r, bk] = asb
                for bk in range(nbank):
                    yps = py_pool.tile([P, 512], dt.float32, space="PSUM",
                                       tag="yps")
                    nc.tensor.matmul(yps[:, :], lhsT=ones1[:], rhs=b_sb[:, :],
                                     start=True, stop=False,
                                     skip_group_check=True)
                    for tl4 in range(min(4, ntl - bk * 4)):
                        c0 = tl4 * P
                        for r in range(3):
                            nc.tensor.matmul(
                                yps[:, c0:c0 + P],
                                lhsT=aggsb[r, bk][:, c0:c0 + P],
                                rhs=W_sb[r][:, :],
                                start=False, stop=False,
                                skip_group_check=True)
                    neg = ypool.tile([P, 512], dt.float32, tag="neg")
                    nc.vector.tensor_scalar_min(neg[:], yps[:, :], 0.0)
                    ysb = ypool.tile([P, 512], dt.float32, tag="ysb")
                    nc.vector.scalar_tensor_tensor(
                        out=ysb[:], in0=neg[:], scalar=am1[:, :1],
                        in1=yps[:, :],
                        op0=mybir.AluOpType.mult,
                        op1=mybir.AluOpType.add)
                    for tl4 in range(min(4, ntl - bk * 4)):
                        t = t0 + bk * 4 + tl4
                        nc.sync.dma_start(
                            y_t[t * P:(t + 1) * P, :],
                            ysb[:, tl4 * P:(tl4 + 1) * P])

    nc.compile()
    return nc


_NC_CACHE = {}


def _profile_key(profile):
    import hashlib
    h = hashlib.sha256()
    for k in sorted(profile):
        pr = profile[k]
        h.update(repr((k, pr["nstar"], pr["pieces"])).encode())
    return h.hexdigest()


def _run(cfg, inputs, trace=False, trace_kwargs=None):
    from concourse.bass_utils import run_bass_kernel_spmd

    x = np.ascontiguousarray(np.asarray(inputs["x"], dtype=np.float32))
    profile, percore = _schedule(cfg, inputs)
    key = (cfg.N, cfg.TGT, _profile_key(profile))
    nc = _NC_CACHE.get(key)
    if nc is None:
        nc = _build_bass(cfg, profile)
        _NC_CACHE.clear()
        _NC_CACHE[key] = nc

    x4h = x.astype(np.float16).reshape(cfg.NQ, NPH * F)
    bsum = (np.asarray(inputs["b0"]) + np.asarray(inputs["b1"])
            + np.asarray(inputs["b2"])).astype(np.float16)
    bvec = np.tile(bsum, 4)[None, :]
    in_maps = []
    for c in range(N_CORES):
        m = {"x4h": x4h, "bvec": bvec,
             "prelu_a": np.asarray(inputs["prelu_a"], dtype=np.float32),
             "cnts": percore[c]["cnts"]}
        for r in range(3):
            m[f"W{r}"] = np.asarray(inputs[f"W{r}"],
                                    dtype=np.float32).astype(np.float16)
        for ph in range(NPH):
            m[f"idx{ph}"] = percore[c][ph]["idx"]
            m[f"wsel{ph}"] = percore[c][ph]["wsel"]
        in_maps.append(m)

    res = run_bass_kernel_spmd(nc, in_maps, core_ids=list(range(N_CORES)),
                               trace=trace, **(trace_kwargs or {}))
    y = np.concatenate(
        [res.results[c]["y"][:cfg.SHARD] for c in range(N_CORES)], axis=0)
    return y, res


def kernel(**inputs) -> np.ndarray:
    cfg = Cfg()
    y, _ = _run(cfg, inputs)
    return y.astype(np.float32)


if __name__ == "__main__":
    pass


# revision 23
# speedup vs baseline: 2.0436x; 1.1671x over previous
"""TRN2 Bass kernel v3 for nn_NodeEmbedding (3-relation GraphConv + PReLU).

y = PReLU( sum_r (D_in^-1/2 A_r D_out^-1/2 x) W_r + b_r )

Design (per core, 12500 dst nodes = 98 tiles of 128):
  - Edges partitioned by dst-owner core; per (phase=src%4, tile-group) the
    needed x rows for ALL 3 relations are fetched with one dma_gather
    (int16 idx = src//4 into a [25000, 512] fp16 view of x).  Merging the
    relations into one gather (52 instead of 156 instructions) amortizes
    the Q7 descriptor-generation fixed cost, which is the kernel's
    critical path (~8ns/descriptor on one Q7 core pair).
  - Per-core padding slots sit at the bucket tail with idx -1 and
    num_idxs_reg loaded from a per-core count table: the Q7 ucode trims
    trailing negatives and the decode reserves ring space by the register,
    so padding costs no descriptor time on cores with fewer edges than the
    SPMD-common max.  Within a bucket, edges sharing (src, relation, tile)
    share one gather slot.
  - Routing/weighting/aggregation on the PE: for each 128-slot block and
    each (tile, r) it touches, matmul(lhsT=G_block[128 slots x 128 f],
    rhs=Wsel[128 slots x 128 dst]) accumulates aggT_r[f, dst] in PSUM.
    Wsel holds w_e = ns[src]*nd[dst] at (slot, dst%128), zeros elsewhere
    (host-precomputed fp16, per-core data; piece list is the union of all
    cores' occupied cells so the instruction stream is SPMD-common).
  - Phase B inline per tile group: aggT (PSUM) -> SBUF fp16, then
    y = sum_r aggT_r^T W_r + b via PE, PReLU on DVE, sequential store.
"""

import numpy as np
from contextlib import ExitStack

P = 128
F = 128
N_CORES = 8

NPH = 4  # src phases (int16 gather idx = src//4 < 25000)
USE_TRIM = False  # per-core num_idxs_reg trim of bucket tail padding


class Cfg:
    def __init__(self, n_nodes=100000, tg_tiles=8):
        self.N = n_nodes
        self.SHARD = n_nodes // N_CORES
        self.TILES = (self.SHARD + P - 1) // P
        self.NQ = n_nodes // NPH  # quad rows
        self.TGT = tg_tiles
        self.NGRP = (self.TILES + tg_tiles - 1) // tg_tiles


def _schedule(cfg, inputs):
    """Host preprocessing (index-side only).

    Returns (profile, percore) where
      profile[(ph, g)] = dict(nstar=int, pieces=[(b, tl, r), ...])
      percore[c][ph] = dict(idx=[128, cols] int16, wsel=[128, cols] fp16)
    tl = tile index local to the group (0..TGT-1).
    """
    N, SHARD, TILES, TGT, NGRP = cfg.N, cfg.SHARD, cfg.TILES, cfg.TGT, cfg.NGRP
    ns, nd = [], []
    for r in range(3):
        src = np.asarray(inputs[f"src{r}"]).astype(np.int64)
        dst = np.asarray(inputs[f"dst{r}"]).astype(np.int64)
        deg_out = np.bincount(src, minlength=N)
        deg_in = np.bincount(dst, minlength=N)
        ns.append((1.0 / np.sqrt(np.maximum(deg_out, 1))).astype(np.float32))
        nd.append((1.0 / np.sqrt(np.maximum(deg_in, 1))).astype(np.float32))

    # per core: merged edge arrays sorted by (ph, g, r, tile, q); within a
    # bucket (ph, g), edges sharing (r, tile, src) share one gather slot
    NQ = cfg.NQ
    bdata = {}
    cnt = np.zeros((N_CORES, NPH, NGRP), np.int64)
    for c in range(N_CORES):
        qs, tls, dms, ws, phs, rs = [], [], [], [], [], []
        for r in range(3):
            src = np.asarray(inputs[f"src{r}"]).astype(np.int64)
            dst = np.asarray(inputs[f"dst{r}"]).astype(np.int64)
            m = (dst // SHARD) == c
            s, d = src[m], dst[m] - c * SHARD
            w = ns[r][s] * nd[r][dst[m]]
            qs.append(s // NPH)
            phs.append(s % NPH)
            tls.append(d // P)
            dms.append(d % P)
            ws.append(w)
            rs.append(np.full(s.shape[0], r, np.int64))
        q = np.concatenate(qs)
        ph = np.concatenate(phs)
        tile = np.concatenate(tls)
        dm = np.concatenate(dms)
        w = np.concatenate(ws).astype(np.float32)
        r_ = np.concatenate(rs)
        g = tile // TGT
        bid = ph * NGRP + g
        order = np.argsort(bid, kind="stable")
        bid, q, tile, dm, w, r_ = (a[order] for a in (bid, q, tile, dm, w, r_))
        bounds = np.searchsorted(bid, np.arange(NPH * NGRP + 1))
        for ph_ in range(NPH):
            for g_ in range(NGRP):
                e0, e1 = bounds[ph_ * NGRP + g_], bounds[ph_ * NGRP + g_ + 1]
                tl = tile[e0:e1] - g_ * TGT
                # slot key sorts by (r, tile, q)
                ek = (r_[e0:e1] * TGT + tl) * NQ + q[e0:e1]
                uk, inv = np.unique(ek, return_inverse=True)
                cnt[c, ph_, g_] = len(uk)
                bdata[c, ph_, g_] = (uk, inv, dm[e0:e1], w[e0:e1])

    nstar = ((cnt.max(0) + 15) // 16) * 16  # [NPH, NGRP]

    profile = {}
    percore = [dict() for _ in range(N_CORES)]
    keyspace = TGT * 3
    for ph in range(NPH):
        for g in range(NGRP):
            nst = int(nstar[ph, g])
            # union of occupied (block, tl, r) cells over cores; cell key
            # encodes (block, tl, r) so sorted keys give (b, tl, r) order
            core_info = []
            allkeys = []
            for c in range(N_CORES):
                uk, inv, dmm, wm = bdata[c, ph, g]
                tlr = uk // NQ
                r_of = tlr // TGT
                tl_of = tlr % TGT
                blk_of = np.arange(len(uk)) // P
                cell_of_slot = blk_of * keyspace + tl_of * 3 + r_of
                cellkey = cell_of_slot[inv]
                allkeys.append(np.unique(cellkey))
                core_info.append((uk % NQ, inv, cellkey, dmm, wm))
            ukeys = np.unique(np.concatenate(allkeys))
            pieces = [(int(k) // keyspace, (int(k) % keyspace) // 3,
                       int(k) % 3) for k in ukeys]
            parr = np.zeros(int(ukeys[-1]) + 1 if len(ukeys) else 1, np.int64)
            parr[ukeys] = np.arange(len(ukeys))
            profile[ph, g] = dict(nstar=nst, pieces=pieces)
            for c in range(N_CORES):
                q_of_slot, inv, cellkey, dmm, wm = core_info[c]
                idx_flat = np.full(nst, -1 if USE_TRIM else 0, np.int16)
                idx_flat[:len(q_of_slot)] = q_of_slot.astype(np.int16)
                wsel32 = np.zeros((P, max(len(pieces), 1) * P), np.float32)
                np.add.at(wsel32, (inv % P, parr[cellkey] * P + dmm), wm)
                percore[c].setdefault(ph, {})[g] = (
                    idx_flat, wsel32.astype(np.float16))

    # concatenate per-ph idx and wsel arrays with per-g column offsets
    for c in range(N_CORES):
        for ph in range(NPH):
            idxs = []
            wsels = []
            for g in range(NGRP):
                idx_flat, wsel = percore[c][ph][g]
                nst = idx_flat.shape[0]
                wrap = np.zeros((16, nst // 16), np.int16)
                ar = np.arange(nst)
                wrap[ar % 16, ar // 16] = idx_flat
                idxs.append(wrap)
                wsels.append(wsel)
            idx_cat = np.concatenate(idxs, axis=1)
            percore[c][ph] = dict(
                idx=np.tile(idx_cat, (8, 1)),
                wsel=np.concatenate(wsels, axis=1))
        percore[c]["cnts"] = cnt[c].reshape(1, NPH * NGRP).astype(np.int32)
    return profile, percore


def _build_bass(cfg, profile):
    import concourse.bass as bass
    import concourse.bacc as bacc
    import concourse.tile as tile
    import concourse.mybir as mybir

    TILES, SHARD, TGT, NGRP = cfg.TILES, cfg.SHARD, cfg.TGT, cfg.NGRP
    dt = mybir.dt

    nc = bacc.Bacc("TRN2", target_bir_lowering=False, debug=False,
                   num_devices=N_CORES, num_swdge_queues=4)

    x4h = nc.dram_tensor("x4h", [cfg.NQ, NPH * F], dt.float16,
                         kind="ExternalInput")
    idx_t, wsel_t = {}, {}
    idxcols = {ph: sum(profile[ph, g]["nstar"] for g in range(NGRP)) // 16
               for ph in range(NPH)}
    wselcols = {ph: sum(max(len(profile[ph, g]["pieces"]), 1)
                        for g in range(NGRP)) * P for ph in range(NPH)}
    for ph in range(NPH):
        idx_t[ph] = nc.dram_tensor(f"idx{ph}", [P, idxcols[ph]], dt.int16,
                                   kind="ExternalInput")
        wsel_t[ph] = nc.dram_tensor(f"wsel{ph}", [P, wselcols[ph]],
                                    dt.float16, kind="ExternalInput")
    W_t = [nc.dram_tensor(f"W{r}", [F, F], dt.float16, kind="ExternalInput")
           for r in range(3)]
    b_t = nc.dram_tensor("bvec", [1, 512], dt.float16, kind="ExternalInput")
    pa_t = nc.dram_tensor("prelu_a", [1], dt.float32, kind="ExternalInput")
    cnt_t = nc.dram_tensor("cnts", [1, NPH * NGRP], dt.int32,
                           kind="ExternalInput")
    y_t = nc.dram_tensor("y", [TILES * P, F], dt.float32,
                         kind="ExternalOutput")

    max_nb = 0
    max_pieces = 1
    for key, pr in profile.items():
        max_nb = max(max_nb, (pr["nstar"] + P - 1) // P)
        max_pieces = max(max_pieces, len(pr["pieces"]))

    with tile.TileContext(nc) as tc:
        with ExitStack() as ctx:
            cpool = ctx.enter_context(tc.tile_pool(name="const", bufs=1))
            ipool = ctx.enter_context(tc.tile_pool(name="idx", bufs=4))
            gpool = ctx.enter_context(tc.tile_pool(name="g", bufs=3))
            wpool = ctx.enter_context(tc.tile_pool(name="wsel", bufs=2))
            apool = ctx.enter_context(tc.tile_pool(name="aggsb", bufs=2))
            ypool = ctx.enter_context(tc.tile_pool(name="y", bufs=2))

            # constants
            W_sb = []
            for r in range(3):
                w_ = cpool.tile([F, F], dt.float16, tag=f"W{r}")
                nc.sync.dma_start(w_[:], W_t[r][:, :])
                W_sb.append(w_)
            b_sb = cpool.tile([1, 512], dt.float16)
            nc.sync.dma_start(b_sb[:], b_t[:, :])
            ones1 = cpool.tile([1, P], dt.float16)
            nc.vector.memset(ones1[:], 1.0)
            ones1f = cpool.tile([1, P], dt.float32)
            nc.vector.memset(ones1f[:], 1.0)
            zvec = cpool.tile([P, 512], dt.float16)
            nc.vector.memset(zvec[:], 0.0)
            pa_sb = cpool.tile([1, 1], dt.float32)
            nc.sync.dma_start(pa_sb[:], pa_t[None, :])
            cnt_sb = cpool.tile([1, NPH * NGRP], dt.int32)
            nc.sync.dma_start(cnt_sb[:], cnt_t[:, :])
            am1 = cpool.tile([P, 1], dt.float32)
            with tc.tile_pool(name="ppsum", bufs=1, space="PSUM") as ppool:
                pa_ps = ppool.tile([P, 1], dt.float32, space="PSUM")
                nc.tensor.matmul(pa_ps[:], lhsT=ones1f[:], rhs=pa_sb[:],
                                 start=True, stop=True)
                nc.vector.tensor_scalar_add(am1[:], pa_ps[:], -1.0)

            pagg = ctx.enter_context(
                tc.tile_pool(name="pagg", bufs=1, space="PSUM"))
            py_pool = ctx.enter_context(
                tc.tile_pool(name="py", bufs=2, space="PSUM"))

            # zero G pool slots once: pieces may read rows no gather wrote
            # (per-core counts < common max); raw SBUF can be NaN patterns
            # and PE NaN*0 = NaN.
            for _ in range(3):
                gz = gpool.tile([P, max_nb, F], dt.float16, tag="G")
                nc.vector.memset(gz[:], 0.0)

            # TRIM_REGS: preload per-bucket runtime counts into Pool
            # registers so no sequencer op sits between gathers (the Pool
            # engine's exec window is what lets gathers on different SWDGE
            # queues overlap on different Q7 core pairs)
            cnt_reg = {}
            if USE_TRIM:
                for g in range(NGRP):
                    for ph in range(NPH):
                        k = ph * NGRP + g
                        cnt_reg[ph, g] = nc.gpsimd.value_load(
                            cnt_sb[0:1, k:k + 1], min_val=0,
                            max_val=profile[ph, g]["nstar"])

            idx_off = {ph: 0 for ph in range(NPH)}
            wsel_off = {ph: 0 for ph in range(NPH)}
            for g in range(NGRP):
                t0 = g * TGT
                t1 = min(t0 + TGT, TILES)
                ntl = t1 - t0
                nbank = (ntl + 3) // 4
                aggps = {}
                for r in range(3):
                    for bk in range(nbank):
                        ps = pagg.tile([P, 512], dt.float32, space="PSUM",
                                       tag=f"agg{r}_{bk}")
                        # zero the bank and set accumulate bits everywhere
                        nc.tensor.matmul(ps[:, :], lhsT=zvec[:, 0:P],
                                         rhs=zvec[:, 0:512], start=True,
                                         stop=False, skip_group_check=True)
                        aggps[r, bk] = ps
                for ph in range(NPH):
                    pr = profile[ph, g]
                    n = pr["nstar"]
                    pieces = pr["pieces"]
                    o16 = idx_off[ph]
                    po = wsel_off[ph]
                    idx_off[ph] += n // 16
                    wsel_off[ph] += max(len(pieces), 1) * P
                    if n == 0:
                        continue
                    nb = (n + P - 1) // P
                    it = ipool.tile([P, n // 16], dt.int16, tag="idx")
                    nc.sync.dma_start(it[:], idx_t[ph][:, o16:o16 + n // 16])
                    G = gpool.tile([P, max_nb, F], dt.float16, tag="G")
                    nc.gpsimd.dma_gather(
                        out_ap=G[:, 0:nb, :],
                        in_ap=x4h[:, ph * F:(ph + 1) * F],
                        idxs_ap=it[:, :],
                        num_idxs=n,
                        num_idxs_reg=cnt_reg[ph, g] if USE_TRIM else n,
                        elem_size=F,
                        elem_step=NPH * F,
                        single_packet=(n <= 1024),
                        queue_num=(g * NPH + ph) % 4)
                    if pieces:
                        wse = wpool.tile([P, max_pieces * P], dt.float16,
                                         tag="wsel")
                        nc.sync.dma_start(
                            wse[:, 0:len(pieces) * P],
                            wsel_t[ph][:, po:po + len(pieces) * P])
                        for k, (b, tl, r) in enumerate(pieces):
                            bk, c0 = tl // 4, (tl % 4) * P
                            nc.tensor.matmul(
                                aggps[r, bk][:, c0:c0 + P],
                                lhsT=G[:, b, :],
                                rhs=wse[:, k * P:(k + 1) * P],
                                start=False, stop=False,
                                skip_group_check=True)
                # ---- phase B for this tile group ----
                aggsb = {}
                for r in range(3):
                    for bk in range(nbank):
                        asb = apool.tile([P, 512], dt.float16,
                                         tag=f"as{r}_{bk}")
                        nc.vector.tensor_copy(asb[:], aggps[r, bk][:, :])
                        aggsb[r, bk] = asb
                for bk in range(nbank):
                    yps = py_pool.tile([P, 512], dt.float32, space="PSUM",
                                       tag="yps")
                    nc.tensor.matmul(yps[:, :], lhsT=ones1[:], rhs=b_sb[:, :],
                                     start=True, stop=False,
                                     skip_group_check=True)
                    for tl4 in range(min(4, ntl - bk * 4)):
                        c0 = tl4 * P
                        for r in range(3):
                            nc.tensor.matmul(
                                yps[:, c0:c0 + P],
                                lhsT=aggsb[r, bk][:, c0:c0 + P],
                                rhs=W_sb[r][:, :],
                                start=False, stop=False,
                                skip_group_check=True)
                    neg = ypool.tile([P, 512], dt.float32, tag="neg")
                    nc.vector.tensor_scalar_min(neg[:], yps[:, :], 0.0)
                    ysb = ypool.tile([P, 512], dt.float32, tag="ysb")
                    nc.vector.scalar_tensor_tensor(
                        out=ysb[:], in0=neg[:], scalar=am1[:, :1],
                        in1=yps[:, :],
                        op0=mybir.AluOpType.mult,
                        op1=mybir.AluOpType.add)
                    for tl4 in range(min(4, ntl - bk * 4)):
                        t = t0 + bk * 4 + tl4
                        nc.sync.dma_start(
                            y_t[t * P:(t + 1) * P, :],
                            ysb[:, tl4 * P:(tl4 + 1) * P])

    nc.compile()
    return nc


_NC_CACHE = {}


def _profile_key(profile):
    import hashlib
    h = hashlib.sha256()
    for k in sorted(profile):
        pr = profile[k]
        h.update(repr((k, pr["nstar"], pr["pieces"])).encode())
    return h.hexdigest()


def _run(cfg, inputs, trace=False, trace_kwargs=None):
    from concourse.bass_utils import run_bass_kernel_spmd

    x = np.ascontiguousarray(np.asarray(inputs["x"], dtype=np.float32))
    profile, percore = _schedule(cfg, inputs)
    key = (cfg.N, cfg.TGT, _profile_key(profile))
    nc = _NC_CACHE.get(key)
    if nc is None:
        nc = _build_bass(cfg, profile)
        _NC_CACHE.clear()
        _NC_CACHE[key] = nc

    x4h = x.astype(np.float16).reshape(cfg.NQ, NPH * F)
    bsum = (np.asarray(inputs["b0"]) + np.asarray(inputs["b1"])
            + np.asarray(inputs["b2"])).astype(np.float16)
    bvec = np.tile(bsum, 4)[None, :]
    in_maps = []
    for c in range(N_CORES):
        m = {"x4h": x4h, "bvec": bvec,
             "prelu_a": np.asarray(inputs["prelu_a"], dtype=np.float32),
             "cnts": percore[c]["cnts"]}
        for r in range(3):
            m[f"W{r}"] = np.asarray(inputs[f"W{r}"],
                                    dtype=np.float32).astype(np.float16)
        for ph in range(NPH):
            m[f"idx{ph}"] = percore[c][ph]["idx"]
            m[f"wsel{ph}"] = percore[c][ph]["wsel"]
        in_maps.append(m)

    res = run_bass_kernel_spmd(nc, in_maps, core_ids=list(range(N_CORES)),
                               trace=trace, **(trace_kwargs or {}))
    y = np.concatenate(
        [res.results[c]["y"][:cfg.SHARD] for c in range(N_CORES)], axis=0)
    return y, res


def kernel(**inputs) -> np.ndarray:
    cfg = Cfg()
    y, _ = _run(cfg, inputs)
    return y.astype(np.float32)


if __name__ == "__main__":
    pass
